# revision 1
# baseline (speedup 1.0000x reference)
"""GAT (2-layer graph attention) on 8 Trainium2 NeuronCores.

Node tables (256B rows: features | attention logits) are AllGather'd and
per-edge rows fetched with dma_gather. int16 gather indices cap tables at
32768 rows, so the 102400-row rank space splits into 4 windows; each
(dst-node, window) pair is a degree-sorted "virtual row" producing partial
softmax sums, combined by a second gather round. Softmax max-subtraction is
dropped (shift-invariant; logits are O(1)).
"""

import numpy as np
import ml_dtypes

bf16 = ml_dtypes.bfloat16

# ---------------- problem constants -----------------------------------
N = 100000
E = 1600000
NC = 8
F_IN = 512
H1, D1 = 8, 8
HD1 = H1 * D1
C = 40
NEG_SLOPE = 0.2
EPS = 1e-16

REAL = N // NC
BLOCKS = 100
SHARD = BLOCKS * 128
RANKS = NC * SHARD
WIN = 32768
NWIN = (RANKS + WIN - 1) // WIN
RW = 128                 # bf16 elems per table row (256B)
BATCH_KMAX = 64          # max sum-of-K per edge-gather call
CHUNK = 25               # combine blocks per chunk
ADST_GMAX = 64


def _set_dims(n, e, nc_, f_in, blocks, win, chunk, batch_kmax, adst_gmax):
    """Test hook: reconfigure sizes (must keep n % nc == 0 etc.)."""
    global N, E, NC, F_IN, REAL, BLOCKS, SHARD, RANKS, WIN, NWIN
    global CHUNK, BATCH_KMAX, ADST_GMAX
    N, E, NC, F_IN, BLOCKS, WIN = n, e, nc_, f_in, blocks, win
    CHUNK, BATCH_KMAX, ADST_GMAX = chunk, batch_kmax, adst_gmax
    REAL = N // NC
    SHARD = BLOCKS * 128
    RANKS = NC * SHARD
    NWIN = (RANKS + WIN - 1) // WIN
    assert REAL <= SHARD and WIN <= 32768


def _wrap_idx(flat):
    n = flat.shape[0]
    assert n % 16 == 0
    w16 = flat.reshape(n // 16, 16).T
    return np.tile(w16, (8, 1)).astype(np.int16)


def _pad_rel(w):
    wbase = w * WIN
    wend = min((w + 1) * WIN, RANKS)
    for c in range(NC):
        g0 = c * SHARD + REAL
        g1 = c * SHARD + SHARD - 1
        if g0 >= wbase and g1 < wend:
            return g0 - wbase
    raise AssertionError(f"no ghost row in window {w}")


def preprocess(edge_index):
    src = np.asarray(edge_index[0], np.int64)
    dst = np.asarray(edge_index[1], np.int64)
    loops = np.arange(N, dtype=np.int64)
    src = np.concatenate([src, loops])
    dst = np.concatenate([dst, loops])

    core = dst // REAL
    r_local = dst % REAL
    srcrank = (src // REAL) * SHARD + (src % REAL)
    w = srcrank // WIN
    rel = srcrank - w * WIN

    key = (core * NWIN + w) * REAL + r_local
    deg = np.bincount(key, minlength=NC * NWIN * REAL).reshape(NC, NWIN, REAL)

    vlists = {}
    nnz = np.zeros((NC, NWIN), int)
    for c in range(NC):
        for wi in range(NWIN):
            d = deg[c, wi]
            rs = np.nonzero(d)[0]
            order = np.argsort(-d[rs], kind="stable")
            rs = rs[order]
            vlists[(c, wi)] = (rs, d[rs])
            nnz[c, wi] = len(rs)

    G_w = [max(1, int(np.ceil(nnz[:, wi].max() / 128))) for wi in range(NWIN)]
    K_w = []
    for wi in range(NWIN):
        ks = np.zeros(G_w[wi], int)
        for c in range(NC):
            degs = vlists[(c, wi)][1]
            for g in range(G_w[wi]):
                if g * 128 < len(degs):
                    ks[g] = max(ks[g], degs[g * 128])
        ks = np.maximum(ks, 1)
        K_w.append(ks)

    # K-uniform batches: every group in a batch is padded to the batch max K
    # so per-group DVE ops fuse into single per-batch ops.
    batches_w = []
    for wi in range(NWIN):
        batches = []
        g0 = 0
        while g0 < G_w[wi]:
            kb = int(K_w[wi][g0])          # groups sorted by K desc
            nb = max(1, min(BATCH_KMAX // kb, G_w[wi] - g0))
            g1 = g0 + nb
            K_w[wi][g0:g1] = kb            # pad K uniform within batch
            batches.append((g0, g1, int(kb * nb)))
            g0 = g1
        batches_w.append(batches)

    cumK_w = [np.concatenate([[0], np.cumsum(K_w[wi])]) for wi in range(NWIN)]
    pad_rel = [_pad_rel(wi) for wi in range(NWIN)]

    sortpos = np.full((NC, NWIN, REAL), -1, np.int64)
    for c in range(NC):
        for wi in range(NWIN):
            rs = vlists[(c, wi)][0]
            sortpos[c, wi, rs] = np.arange(len(rs))
    vpos = sortpos[core, w, r_local]
    order = np.argsort(key, kind="stable")
    k_in_row = np.empty(len(key), np.int64)
    sk_ = key[order]
    first = np.concatenate([[True], sk_[1:] != sk_[:-1]])
    starts = np.nonzero(first)[0]
    run_id = np.cumsum(first) - 1
    k_in_row[order] = np.arange(len(key)) - starts[run_id]

    gv = vpos // 128
    pv = vpos % 128

    idx_e_cores, idx_a_cores, idx_c_cores = [], [], []
    for c in range(NC):
        e_parts = []
        m_c = core == c
        for wi in range(NWIN):
            A = np.full((128, int(cumK_w[wi][-1])), pad_rel[wi], np.int64)
            m = m_c & (w == wi)
            col = cumK_w[wi][gv[m]] + k_in_row[m]
            A[pv[m], col] = rel[m]
            for (g0, g1, _sk) in batches_w[wi]:
                c0, c1 = int(cumK_w[wi][g0]), int(cumK_w[wi][g1])
                e_parts.append(_wrap_idx(A[:, c0:c1].T.reshape(-1)))
        idx_e_cores.append(np.concatenate(e_parts, axis=1))

        a_parts = []
        for wi in range(NWIN):
            rs = vlists[(c, wi)][0]
            R_flat = np.zeros(G_w[wi] * 128, np.int64)
            R_flat[: len(rs)] = rs
            R = R_flat.reshape(G_w[wi], 128).T
            g0 = 0
            while g0 < G_w[wi]:
                g1 = min(g0 + ADST_GMAX, G_w[wi])
                a_parts.append(_wrap_idx(R[:, g0:g1].T.reshape(-1)))
                g0 = g1
        idx_a_cores.append(np.concatenate(a_parts, axis=1))

        c_parts = []
        for wi in range(NWIN):
            zr = 128 * G_w[wi]
            Cidx = np.full(SHARD, zr, np.int64)
            rs = vlists[(c, wi)][0]
            vp = np.arange(len(rs))
            Cidx[rs] = (vp % 128) * G_w[wi] + (vp // 128)
            Cm = Cidx.reshape(BLOCKS, 128).T
            for b0 in range(0, BLOCKS, CHUNK):
                b1 = min(b0 + CHUNK, BLOCKS)
                c_parts.append(_wrap_idx(Cm[:, b0:b1].T.reshape(-1)))
        idx_c_cores.append(np.concatenate(c_parts, axis=1))

    struct = dict(
        G_w=G_w, K_w=K_w, batches_w=batches_w, cumK_w=cumK_w,
        idx_e_w=idx_e_cores[0].shape[1], idx_a_w=idx_a_cores[0].shape[1],
        idx_c_w=idx_c_cores[0].shape[1],
    )
    return struct, idx_e_cores, idx_a_cores, idx_c_cores


# -----------------------------------------------------------------------
def build(struct):
    import os
    STAGE = int(os.environ.get("KSTAGE", "5"))
    KEDGE = int(os.environ.get("KEDGE", "3"))
    import concourse.bacc as bacc
    import concourse.mybir as mybir
    import concourse.tile as tile
    from concourse.masks import make_identity

    F32 = mybir.dt.float32
    BF = mybir.dt.bfloat16
    I16 = mybir.dt.int16
    AX = mybir.AxisListType.X
    OP = mybir.AluOpType
    ACT = mybir.ActivationFunctionType

    G_w, K_w, batches_w = struct["G_w"], struct["K_w"], struct["batches_w"]
    cumK_w = struct["cumK_w"]
    KMAX = int(max(max(k) for k in K_w))
    FC = F_IN // 128

    nc = bacc.Bacc("TRN2", target_bir_lowering=False, debug=False,
                   num_devices=NC, num_swdge_queues=4)

    xt = nc.dram_tensor("xt", [F_IN, SHARD], F32, kind="ExternalInput").ap()
    w1 = nc.dram_tensor("w1", [F_IN, HD1], F32, kind="ExternalInput").ap()
    w2 = nc.dram_tensor("w2", [HD1, C], F32, kind="ExternalInput").ap()
    vec_in = {}
    for nm, width in [("atts1", HD1), ("attd1", HD1), ("b1", HD1),
                      ("atts2", C), ("attd2", C), ("b2", C)]:
        vec_in[nm] = nc.dram_tensor(nm, [1, width], F32,
                                    kind="ExternalInput").ap()
    idx_e = nc.dram_tensor("idx_e", [128, struct["idx_e_w"]], I16,
                           kind="ExternalInput").ap()
    idx_a = nc.dram_tensor("idx_a", [128, struct["idx_a_w"]], I16,
                           kind="ExternalInput").ap()
    idx_c = nc.dram_tensor("idx_c", [128, struct["idx_c_w"]], I16,
                           kind="ExternalInput").ap()
    out = nc.dram_tensor("out", [SHARD, C], F32, kind="ExternalOutput").ap()

    rg = [list(range(NC))]
    PT_rows = [128 * G_w[wi] + 1 for wi in range(NWIN)]
    PT_total = sum(PT_rows)
    PT_base = np.concatenate([[0], np.cumsum(PT_rows)]).astype(int)

    with tile.TileContext(nc) as tc:
        with (
            tc.tile_pool(name="dram", bufs=1, space="DRAM") as dpool,
            tc.tile_pool(name="setup", bufs=1) as sup,
            tc.tile_pool(name="psum0", bufs=2, space="PSUM") as psp,
        ):
            Rshard1 = dpool.tile([SHARD, RW], BF, tag="rs1")
            Rshard2 = dpool.tile([SHARD, RW], BF, tag="rs2")
            Rfull1 = dpool.tile([RANKS, RW], BF, tag="rf1")
            Rfull2 = dpool.tile([RANKS, RW], BF, tag="rf2")
            AdstT = dpool.tile([SHARD, RW], BF, tag="adt")
            Ptab1 = dpool.tile([PT_total, RW], BF, tag="pt1")
            Ptab2 = dpool.tile([PT_total, RW], BF, tag="pt2")

            ident = sup.tile([128, 128], F32)
            make_identity(nc, ident[:])
            ones_row = sup.tile([1, 128], F32)
            nc.vector.memset(ones_row[:], 1.0)

            w1_t = sup.tile([128, FC * HD1], F32)
            nc.sync.dma_start(
                w1_t[:].rearrange("p (c n) -> p c n", c=FC),
                w1.rearrange("(c p) n -> p c n", p=128),
            )
            w2_t = sup.tile([128, C], F32)
            nc.sync.dma_start(w2_t[0:HD1, :], w2[:, :])
            nc.sync.dma_start(w2_t[HD1:2 * HD1, :], w2[:, :])

            reps = {}
            for nm in ["atts1", "attd1", "b1", "atts2", "attd2", "b2"]:
                width = HD1 if nm in ("atts1", "attd1", "b1") else C
                v = sup.tile([1, width], F32, tag=f"v_{nm}")
                nc.sync.dma_start(v[:], vec_in[nm][:, :])
                ps = psp.tile([128, width], F32, tag="rep_ps")
                nc.tensor.matmul(out=ps[:], lhsT=ones_row[:], rhs=v[:],
                                 start=True, stop=True)
                r_ = sup.tile([128, width], F32, tag=f"rep_{nm}")
                nc.vector.tensor_copy(r_[:], ps[:])
                reps[nm] = r_

            ghost1 = sup.tile([128, 8], BF)
            nc.vector.memset(ghost1[:], -100.0)
            zrow = sup.tile([1, RW], BF)
            nc.vector.memset(zrow[:], 0.0)
            for wi in range(NWIN):
                zr = int(PT_base[wi]) + 128 * G_w[wi]
                nc.sync.dma_start(Ptab1[:][zr:zr + 1, :], zrow[:])
                nc.sync.dma_start(Ptab2[:][zr:zr + 1, :], zrow[:])

            # ---------------- dense layer 1 ----------------
            with (
                tc.tile_pool(name="d1", bufs=3) as dp,
                tc.tile_pool(name="d1p", bufs=2, space="PSUM") as dpp,
            ):
                for t in range(BLOCKS):
                    xtile = dp.tile([128, FC * 128], F32, tag="x")
                    nc.sync.dma_start(
                        xtile[:].rearrange("p (c n) -> p c n", c=FC),
                        xt.rearrange("(c p) n -> p c n", p=128)[
                            :, :, t * 128:(t + 1) * 128],
                    )
                    hps = dpp.tile([128, HD1], F32, tag="h")
                    for cc in range(FC):
                        nc.tensor.matmul(
                            out=hps[:],
                            lhsT=xtile[:].rearrange(
                                "p (c n) -> p c n", c=FC)[:, cc, :],
                            rhs=w1_t[:].rearrange(
                                "p (c n) -> p c n", c=FC)[:, cc, :],
                            start=(cc == 0), stop=(cc == FC - 1),
                        )
                    row = dp.tile([128, RW], BF, tag="row")
                    nc.vector.memset(row[:, 80:RW], 0.0)
                    nc.vector.tensor_copy(row[:, 0:HD1], hps[:])
                    asrc_f = dp.tile([128, H1], F32, tag="asrcf")
                    adst_f = dp.tile([128, H1], F32, tag="adstf")
                    tmp = dp.tile([128, HD1], F32, tag="tmp")
                    for nm, dst_ap in (("atts1", asrc_f), ("attd1", adst_f)):
                        nc.vector.tensor_tensor(
                            out=tmp[:], in0=hps[:], in1=reps[nm][:],
                            op=OP.mult)
                        nc.vector.tensor_reduce(
                            out=dst_ap[:],
                            in_=tmp[:].rearrange("p (h d) -> p h d", h=H1),
                            axis=AX, op=OP.add)
                    nc.vector.tensor_copy(row[:, 64:72], asrc_f[:])
                    nc.vector.tensor_copy(row[:, 72:80], adst_f[:])
                    nc.sync.dma_start(Rshard1[t * 128:(t + 1) * 128, :],
                                      row[:])
                    arow = dp.tile([128, RW], BF, tag="arow")
                    nc.vector.memset(arow[:, 8:RW], 0.0)
                    nc.vector.tensor_copy(arow[:, 0:8], adst_f[:])
                    nc.sync.dma_start(AdstT[t * 128:(t + 1) * 128, :],
                                      arow[:])

                for r0 in range(REAL, SHARD, 128):
                    r1 = min(r0 + 128, SHARD)
                    nc.sync.dma_start(Rshard1[r0:r1, 64:72],
                                      ghost1[: r1 - r0, :])

            nc.gpsimd.collective_compute(
                "AllGather", OP.bypass, replica_groups=rg,
                ins=[Rshard1.opt()], outs=[Rfull1.opt()])

            # ---------------- edge phase ----------------
            qn = [0, None]

            def edge_phase(Rfull, Ptab, layer):
                e_col = 0
                a_col = 0
                if layer == 1:
                    Hh, Dd, alo, dlo = H1, D1, 64, 0
                else:
                    Hh, Dd, alo, dlo = 1, C, 40, 8
                for wi in range(NWIN):
                    wbase = wi * WIN
                    wrows = min(WIN, RANKS - wbase)
                    Gn = G_w[wi]
                    ecols_w = 8 * sum(sk for (_g0, _g1, sk) in batches_w[wi])
                    acols_w = Gn * 8
                    with (
                        tc.tile_pool(name=f"ad{layer}{wi}", bufs=1) as apool,
                        tc.tile_pool(name=f"eg{layer}{wi}", bufs=4) as gp,
                        tc.tile_pool(name=f"ep{layer}{wi}", bufs=2) as ep,
                        tc.tile_pool(name=f"em{layer}{wi}", bufs=1) as epm,
                        tc.tile_pool(name=f"eb{layer}{wi}", bufs=2) as epb,
                        tc.tile_pool(name=f"ix{layer}{wi}", bufs=1) as ixp,
                    ):
                        ixw = ixp.tile([128, ecols_w], I16, tag="ixw")
                        nc.sync.dma_start(ixw[:], idx_e[:, e_col: e_col + ecols_w])
                        ixaw = ixp.tile([128, acols_w], I16, tag="ixaw")
                        nc.sync.dma_start(ixaw[:], idx_a[:, a_col: a_col + acols_w])
                        ecol_loc = 0
                        acol_loc = 0
                        adstG = apool.tile([128, Gn * RW], BF, tag="adstG")
                        adstG_v = adstG[:].rearrange("p (g e) -> p g e", e=RW)
                        g0 = 0
                        while g0 < Gn:
                            g1 = min(g0 + ADST_GMAX, Gn)
                            nidx = (g1 - g0) * 128
                            _gi = nc.gpsimd.dma_gather(
                                out_ap=adstG_v[:, g0:g1, :],
                                in_ap=AdstT[:, :],
                                idxs_ap=ixaw[:, acol_loc: acol_loc + nidx // 16],
                                num_idxs=nidx, num_idxs_reg=nidx,
                                elem_size=RW, single_packet=False,
                                queue_num=qn[0] % 4)
                            if qn[1] is not None:
                                tile.add_dep_helper(_gi.ins, qn[1].ins, sync=False,
                                                    reason="swdge order")
                            qn[1] = _gi
                            qn[0] += 1
                            a_col += nidx // 16
                            acol_loc += nidx // 16
                            g0 = g1

                        for bidx, (g0, g1, sk) in enumerate(batches_w[wi]):
                            Kb = int(K_w[wi][g0])
                            ng = g1 - g0
                            nidx = 128 * sk
                            ixe = ixw[:, ecol_loc: ecol_loc + nidx // 16]
                            e_col += nidx // 16
                            ecol_loc += nidx // 16
                            G = gp.tile([128, BATCH_KMAX * RW], BF, tag="G")
                            Gv = G[:].rearrange("p (k e) -> p k e", e=RW)
                            Gg = G[:][:, 0:sk * RW].rearrange(
                                "p (g k e) -> p g k e", g=ng, k=Kb)
                            _gi = nc.gpsimd.dma_gather(
                                out_ap=Gv[:, 0:sk, :],
                                in_ap=Rfull[:][wbase: wbase + wrows, :],
                                idxs_ap=ixe,
                                num_idxs=nidx, num_idxs_reg=nidx,
                                elem_size=RW, single_packet=False,
                                queue_num=qn[0] % 4)
                            if qn[1] is not None:
                                tile.add_dep_helper(_gi.ins, qn[1].ins, sync=False,
                                                    reason="swdge order")
                            qn[1] = _gi
                            qn[0] += 1

                            if KEDGE < 2:
                                continue
                            eT = ep.tile([128, BATCH_KMAX * Hh], F32,
                                         tag="eT")
                            eV = eT[:][:, 0:sk * Hh].rearrange(
                                "p (g k h) -> p g k h", g=ng, k=Kb)
                            pT = ep.tile([128, BATCH_KMAX * Hh], BF, tag="pT")
                            pV = pT[:][:, 0:sk * Hh].rearrange(
                                "p (g k h) -> p g k h", g=ng, k=Kb)
                            pb = epb.tile([128, BATCH_KMAX * RW], BF, tag="pb")
                            pbV = pb[:].rearrange("p (g e) -> p g e", e=RW)
                            if layer == 1:
                                nc.vector.memset(pbV[:, 0:ng, 80:RW], 0.0)
                            else:
                                nc.vector.memset(pbV[:, 0:ng, 40:64], 0.0)
                                nc.vector.memset(pbV[:, 0:ng, 66:RW], 0.0)

                            # e = a_src[src] + a_dst (one op per batch)
                            nc.vector.tensor_tensor(
                                out=eV[:, :, :, :],
                                in0=Gg[:, :, :, alo:alo + Hh],
                                in1=adstG_v[:, g0:g1, dlo:dlo + Hh]
                                    .unsqueeze(2)
                                    .to_broadcast([128, ng, Kb, Hh]),
                                op=OP.add)
                            # leaky relu + exp
                            ee = ep.tile([128, BATCH_KMAX * Hh], F32,
                                         tag="ee")
                            nc.vector.tensor_scalar_mul(
                                ee[:, : sk * Hh], eT[:, : sk * Hh], NEG_SLOPE)
                            nc.vector.tensor_tensor(
                                out=eT[:, : sk * Hh], in0=eT[:, : sk * Hh],
                                in1=ee[:, : sk * Hh], op=OP.max)
                            nc.scalar.activation(
                                pT[:, : sk * Hh], eT[:, : sk * Hh], ACT.Exp)

                            # s = sum_k p   (one strided reduce per batch)
                            sW = ep.tile([128, BATCH_KMAX * Hh], F32,
                                         tag="sW")
                            nc.vector.tensor_reduce(
                                out=sW[:, 0: ng * Hh],
                                in_=pV.transpose([0, 1, 3, 2]),
                                axis=AX, op=OP.add)
                            # msg = p * h[src]  (one op; (g,k) dims merge)
                            msg = epm.tile([128, BATCH_KMAX * HD1], F32,
                                           tag="msg")
                            msgV = msg[:][:, 0:sk * Hh * Dd].rearrange(
                                "p (g k f) -> p g k f", g=ng, k=Kb)
                            nc.vector.tensor_tensor(
                                out=msg[:][:, 0:sk * Hh * Dd].rearrange(
                                    "p (k h d) -> p k h d", k=sk, h=Hh),
                                in0=Gv[:, 0:sk, 0:Hh * Dd].rearrange(
                                    "p k (h d) -> p k h d", h=Hh),
                                in1=pT[:][:, 0:sk * Hh].rearrange(
                                    "p (k h) -> p k h", h=Hh)
                                    .unsqueeze(3)
                                    .to_broadcast([128, sk, Hh, Dd]),
                                op=OP.mult)
                            # tree-reduce over k (uniform Kb)
                            kk = Kb
                            while kk > 1:
                                half = kk // 2
                                nc.vector.tensor_tensor(
                                    out=msgV[:, :, 0:half, :],
                                    in0=msgV[:, :, 0:half, :],
                                    in1=msgV[:, :, half:2 * half, :],
                                    op=OP.add)
                                if kk % 2 == 1:
                                    nc.vector.tensor_tensor(
                                        out=msgV[:, :, 0:1, :],
                                        in0=msgV[:, :, 0:1, :],
                                        in1=msgV[:, :, kk - 1:kk, :],
                                        op=OP.add)
                                kk = half
                            # pack partial rows
                            nc.vector.tensor_copy(
                                pbV[:, 0:ng, 0:Hh * Dd],
                                msgV[:, :, 0, :])
                            nc.vector.tensor_copy(
                                pbV[:, 0:ng, 64:64 + 2 * Hh].bitcast(F32),
                                sW[:].rearrange(
                                    "p (g h) -> p g h", h=Hh)[:, 0:ng, :])
                            if KEDGE >= 3:
                                nc.sync.dma_start(
                                    Ptab[:][int(PT_base[wi]):
                                            int(PT_base[wi]) + 128 * Gn, :]
                                    .rearrange("(p g) e -> p g e", p=128)
                                    [:, g0:g1, :],
                                    pbV[:, 0:ng, :])

            if STAGE >= 2:
                edge_phase(Rfull1, Ptab1, 1)

            # ------------- combine helpers -------------
            def combine_chunks(Ptab, body, cp, cxp, tagp):
                ixcw = cxp.tile([128, NWIN * BLOCKS * 8], I16, tag="ixcw")
                nc.sync.dma_start(ixcw[:], idx_c[:, :])
                for b0 in range(0, BLOCKS, CHUNK):
                    b1 = min(b0 + CHUNK, BLOCKS)
                    nb = b1 - b0
                    CWs = []
                    for wi in range(NWIN):
                        nidx = nb * 128
                        off = (wi * BLOCKS + b0) * 128 // 16
                        ixc = ixcw[:, off: off + nidx // 16]
                        CW = cp.tile([128, CHUNK * RW], BF,
                                     tag=f"cw{tagp}{wi}")
                        _gi = nc.gpsimd.dma_gather(
                            out_ap=CW[:].rearrange(
                                "p (b e) -> p b e", e=RW)[:, 0:nb, :],
                            in_ap=Ptab[:][int(PT_base[wi]):
                                          int(PT_base[wi]) + PT_rows[wi], :],
                            idxs_ap=ixc,
                            num_idxs=nidx, num_idxs_reg=nidx,
                            elem_size=RW, single_packet=False,
                            queue_num=qn[0] % 4)
                        if qn[1] is not None:
                            tile.add_dep_helper(_gi.ins, qn[1].ins, sync=False,
                                                reason="swdge order")
                        qn[1] = _gi
                        qn[0] += 1
                        CWs.append(CW[:].rearrange("p (b e) -> p b e", e=RW))
                    body(b0, b1, CWs)

            def add4(cp, CWs, nb, lo, hi, ftag):
                width = hi - lo
                acc = cp.tile([128, CHUNK * width], F32, tag=f"acc{ftag}")
                t0 = cp.tile([128, CHUNK * width], F32, tag=f"t0{ftag}")
                accV = acc[:].rearrange("p (b f) -> p b f", f=width)
                t0V = t0[:].rearrange("p (b f) -> p b f", f=width)
                nc.vector.tensor_tensor(
                    out=accV[:, 0:nb], in0=CWs[0][:, 0:nb, lo:hi],
                    in1=CWs[1][:, 0:nb, lo:hi], op=OP.add)
                if NWIN > 2:
                    nc.vector.tensor_tensor(
                        out=t0V[:, 0:nb], in0=CWs[2][:, 0:nb, lo:hi],
                        in1=CWs[3][:, 0:nb, lo:hi], op=OP.add)
                    nc.vector.tensor_tensor(
                        out=accV[:, 0:nb], in0=accV[:, 0:nb],
                        in1=t0V[:, 0:nb], op=OP.add)
                return accV

            def add4_f32(cp, CWs, nb, lo, nf, ftag):
                acc = cp.tile([128, CHUNK * nf], F32, tag=f"acs{ftag}")
                t0 = cp.tile([128, CHUNK * nf], F32, tag=f"ts{ftag}")
                accV = acc[:].rearrange("p (b f) -> p b f", f=nf)
                t0V = t0[:].rearrange("p (b f) -> p b f", f=nf)
                nc.vector.tensor_tensor(
                    out=accV[:, 0:nb],
                    in0=CWs[0][:, 0:nb, lo:lo + 2 * nf].bitcast(F32),
                    in1=CWs[1][:, 0:nb, lo:lo + 2 * nf].bitcast(F32),
                    op=OP.add)
                if NWIN > 2:
                    nc.vector.tensor_tensor(
                        out=t0V[:, 0:nb],
                        in0=CWs[2][:, 0:nb, lo:lo + 2 * nf].bitcast(F32),
                        in1=CWs[3][:, 0:nb, lo:lo + 2 * nf].bitcast(F32),
                        op=OP.add)
                    nc.vector.tensor_tensor(
                        out=accV[:, 0:nb], in0=accV[:, 0:nb],
                        in1=t0V[:, 0:nb], op=OP.add)
                return accV

            # ---------------- combine L1 + dense layer 2 ----------------
            if STAGE >= 3:
              with (
                  tc.tile_pool(name="c1", bufs=2) as cp,
                  tc.tile_pool(name="c1x", bufs=2) as cxp,
                  tc.tile_pool(name="c1p", bufs=2, space="PSUM") as cpp,
              ):
                  def c1_body(b0, b1, CWs):
                      nb = b1 - b0
                      UcV = add4(cp, CWs, nb, 0, HD1, "u1")
                      ScV = add4_f32(cp, CWs, nb, 64, H1, "s1")
                      Ucf = UcV.rearrange("p b f -> p (b f)")
                      Scf = ScV.rearrange("p b f -> p (b f)")
                      rinv = cp.tile([128, CHUNK * H1], F32, tag="rinv")
                      nc.vector.tensor_scalar(
                          out=rinv[:, 0:nb * H1], in0=Scf, scalar1=EPS,
                          scalar2=None, op0=OP.add)
                      nc.vector.reciprocal(rinv[:, 0:nb * H1],
                                           rinv[:, 0:nb * H1])
                      o1c = cp.tile([128, CHUNK * HD1], F32, tag="o1c")
                      nc.vector.tensor_tensor(
                          out=o1c[:][:, 0:nb * HD1].rearrange(
                              "p (b h d) -> p b h d", b=nb, h=H1),
                          in0=UcV.rearrange("p b (h d) -> p b h d", h=H1),
                          in1=rinv[:][:, 0:nb * H1].rearrange(
                              "p (b h) -> p b h", b=nb).unsqueeze(3)
                              .to_broadcast([128, nb, H1, D1]),
                          op=OP.mult)
                      nc.vector.tensor_tensor(
                          out=o1c[:][:, 0:nb * HD1].rearrange(
                              "p (b f) -> p b f", b=nb),
                          in0=o1c[:][:, 0:nb * HD1].rearrange(
                              "p (b f) -> p b f", b=nb),
                          in1=reps["b1"][:].unsqueeze(1)
                              .to_broadcast([128, nb, HD1]),
                          op=OP.add)
                      of = o1c[:, 0:nb * HD1]
                      mn = cp.tile([128, CHUNK * HD1], F32, tag="mn")
                      nc.vector.tensor_scalar(
                          out=mn[:, 0:nb * HD1], in0=of, scalar1=0.0,
                          scalar2=None, op0=OP.min)
                      ex = cp.tile([128, CHUNK * HD1], F32, tag="ex")
                      nc.scalar.activation(ex[:, 0:nb * HD1], mn[:, 0:nb * HD1],
                                           ACT.Exp)
                      nc.vector.tensor_scalar(
                          out=of, in0=of, scalar1=0.0, scalar2=None, op0=OP.max)
                      nc.vector.tensor_tensor(
                          out=of, in0=of, in1=ex[:, 0:nb * HD1], op=OP.add)
                      nc.vector.tensor_scalar(
                          out=of, in0=of, scalar1=-1.0, scalar2=None, op0=OP.add)
                      # h2 = elu @ W2 : transpose 2 blocks at a time
                      h2c = cp.tile([128, CHUNK * C], F32, tag="h2c")
                      h2cV = h2c[:][:, 0:nb * C].rearrange(
                          "p (b f) -> p b f", b=nb)
                      for bp in range(0, nb, 2):
                          npair = min(2, nb - bp)
                          tp = cpp.tile([128, 128], F32, tag="tp")
                          nc.tensor.transpose(
                              out=tp[0:npair * HD1, :],
                              in_=o1c[:, bp * HD1:(bp + npair) * HD1],
                              identity=ident[:])
                          eT_ = cp.tile([128, 128], F32, tag="eT2")
                          nc.vector.tensor_copy(eT_[0:npair * HD1, :],
                                                tp[0:npair * HD1, :])
                          for j in range(npair):
                              h2p = cpp.tile([128, C], F32, tag="h2p")
                              nc.tensor.matmul(
                                  out=h2p[:],
                                  lhsT=eT_[j * HD1:(j + 1) * HD1, :],
                                  rhs=w2_t[j * HD1:(j + 1) * HD1, :],
                                  start=True, stop=True)
                              nc.vector.tensor_copy(
                                  h2c[:, (bp + j) * C:(bp + j + 1) * C],
                                  h2p[:])
                      # attention logits for layer 2 (chunk-wide)
                      tmp2 = cp.tile([128, CHUNK * C], F32, tag="tmp2")
                      a2s = cp.tile([128, CHUNK], F32, tag="a2s")
                      a2d = cp.tile([128, CHUNK], F32, tag="a2d")
                      for nm, dst in (("atts2", a2s), ("attd2", a2d)):
                          nc.vector.tensor_tensor(
                              out=tmp2[:][:, 0:nb * C].rearrange(
                                  "p (b f) -> p b f", b=nb),
                              in0=h2cV,
                              in1=reps[nm][:].unsqueeze(1)
                                  .to_broadcast([128, nb, C]),
                              op=OP.mult)
                          nc.vector.tensor_reduce(
                              out=dst[:, 0:nb],
                              in_=tmp2[:][:, 0:nb * C].rearrange(
                                  "p (b f) -> p b f", b=nb),
                              axis=AX, op=OP.add)
                      row2c = cp.tile([128, CHUNK * RW], BF, tag="row2c")
                      r2V = row2c[:].rearrange("p (b e) -> p b e", e=RW)
                      nc.vector.memset(r2V[:, 0:nb, 42:RW], 0.0)
                      nc.vector.tensor_copy(r2V[:, 0:nb, 0:C], h2cV)
                      nc.vector.tensor_copy(
                          r2V[:, 0:nb, 40:41],
                          a2s[:][:, 0:nb].unsqueeze(2))
                      nc.vector.tensor_copy(
                          r2V[:, 0:nb, 41:42],
                          a2d[:][:, 0:nb].unsqueeze(2))
                      nc.sync.dma_start(
                          Rshard2[b0 * 128:b1 * 128, :].rearrange(
                              "(b p) e -> p b e", p=128),
                          r2V[:, 0:nb, :])
                      adr = cp.tile([128, CHUNK], BF, tag="adr")
                      nc.vector.tensor_copy(adr[:, 0:nb], a2d[:, 0:nb])
                      nc.sync.dma_start(
                          AdstT[b0 * 128:b1 * 128, 8:9].rearrange(
                              "(b p) e -> p b e", p=128),
                          adr[:][:, 0:nb].unsqueeze(2))

                  combine_chunks(Ptab1, c1_body, cp, cxp, "a")

                  gz = cp.tile([128, 42], BF, tag="gz")
                  nc.vector.memset(gz[:, 0:40], 0.0)
                  nc.vector.memset(gz[:, 40:41], -100.0)
                  nc.vector.memset(gz[:, 41:42], 0.0)
                  for r0 in range(REAL, SHARD, 128):
                      r1 = min(r0 + 128, SHARD)
                      nc.sync.dma_start(Rshard2[r0:r1, 0:42],
                                        gz[: r1 - r0, :])

            if STAGE >= 4:
                nc.gpsimd.collective_compute(
                    "AllGather", OP.bypass, replica_groups=rg,
                    ins=[Rshard2.opt()], outs=[Rfull2.opt()])
                edge_phase(Rfull2, Ptab2, 2)

            # ---------------- combine L2 + log_softmax ----------------
            if STAGE >= 5:
              with (
                  tc.tile_pool(name="c2", bufs=2) as cp2,
                  tc.tile_pool(name="c2x", bufs=2) as cxp2,
              ):
                  def c2_body(b0, b1, CWs):
                      nb = b1 - b0
                      UcV = add4(cp2, CWs, nb, 0, C, "u2")
                      ScV = add4_f32(cp2, CWs, nb, 64, 1, "s2")
                      rinv = cp2.tile([128, CHUNK], F32, tag="rinv2")
                      nc.vector.tensor_scalar(
                          out=rinv[:, 0:nb],
                          in0=ScV.rearrange("p b f -> p (b f)"),
                          scalar1=EPS, scalar2=None, op0=OP.add)
                      nc.vector.reciprocal(rinv[:, 0:nb], rinv[:, 0:nb])
                      o2c = cp2.tile([128, CHUNK * C], F32, tag="o2c")
                      o2V = o2c[:][:, 0:nb * C].rearrange(
                          "p (b f) -> p b f", b=nb)
                      nc.vector.tensor_tensor(
                          out=o2V, in0=UcV,
                          in1=rinv[:][:, 0:nb].unsqueeze(2)
                              .to_broadcast([128, nb, C]),
                          op=OP.mult)
                      nc.vector.tensor_tensor(
                          out=o2V, in0=o2V,
                          in1=reps["b2"][:].unsqueeze(1)
                              .to_broadcast([128, nb, C]),
                          op=OP.add)
                      mx = cp2.tile([128, CHUNK], F32, tag="mx")
                      nc.vector.tensor_reduce(
                          out=mx[:, 0:nb], in_=o2V, axis=AX, op=OP.max)
                      nc.vector.tensor_tensor(
                          out=o2V, in0=o2V,
                          in1=mx[:][:, 0:nb].unsqueeze(2)
                              .to_broadcast([128, nb, C]),
                          op=OP.subtract)
                      ex2 = cp2.tile([128, CHUNK * C], F32, tag="ex2")
                      nc.scalar.activation(ex2[:, 0:nb * C], o2c[:, 0:nb * C],
                                           ACT.Exp)
                      ss = cp2.tile([128, CHUNK], F32, tag="ss")
                      nc.vector.tensor_reduce(
                          out=ss[:, 0:nb],
                          in_=ex2[:][:, 0:nb * C].rearrange(
                              "p (b f) -> p b f", b=nb),
                          axis=AX, op=OP.add)
                      nc.scalar.activation(ss[:, 0:nb], ss[:, 0:nb], ACT.Ln)
                      nc.vector.tensor_tensor(
                          out=o2V, in0=o2V,
                          in1=ss[:][:, 0:nb].unsqueeze(2)
                              .to_broadcast([128, nb, C]),
                          op=OP.subtract)
                      nc.sync.dma_start(
                          out[b0 * 128:b1 * 128, :].rearrange(
                              "(b p) c -> p b c", p=128),
                          o2V)

                  combine_chunks(Ptab2, c2_body, cp2, cxp2, "b")

    nc.compile()
    return nc


_CACHE = {}


def _in_maps(inputs, idx_e, idx_a, idx_c):
    x = np.asarray(inputs["x"], np.float32)
    maps = []
    for c in range(NC):
        xs = np.zeros((F_IN, SHARD), np.float32)
        xs[:, :REAL] = x[c * REAL:(c + 1) * REAL].T
        maps.append({
            "xt": xs,
            "w1": np.asarray(inputs["W1"], np.float32),
            "w2": np.asarray(inputs["W2"], np.float32),
            "atts1": np.asarray(inputs["att_src1"], np.float32).reshape(1, HD1),
            "attd1": np.asarray(inputs["att_dst1"], np.float32).reshape(1, HD1),
            "b1": np.asarray(inputs["b1"], np.float32).reshape(1, HD1),
            "atts2": np.asarray(inputs["att_src2"], np.float32).reshape(1, C),
            "attd2": np.asarray(inputs["att_dst2"], np.float32).reshape(1, C),
            "b2": np.asarray(inputs["b2"], np.float32).reshape(1, C),
            "idx_e": idx_e[c], "idx_a": idx_a[c], "idx_c": idx_c[c],
        })
    return maps


def kernel(**inputs):
    from concourse import bass_utils

    struct, idx_e, idx_a, idx_c = preprocess(inputs["edge_index"])
    key = (struct["idx_e_w"], struct["idx_a_w"], struct["idx_c_w"],
           tuple(struct["G_w"]))
    if key not in _CACHE:
        _CACHE[key] = build(struct)
    nc = _CACHE[key]

    maps = _in_maps(inputs, idx_e, idx_a, idx_c)
    res = bass_utils.run_bass_kernel_spmd(nc, maps, core_ids=list(range(NC)))
    out = np.concatenate(
        [res.results[c]["out"][:REAL] for c in range(NC)], axis=0)
    return out.astype(np.float32)


if __name__ == "__main__":
    rng = np.random.default_rng(0)
    ei = np.stack([rng.integers(0, N, E), rng.integers(0, N, E)])
    struct, *_ = preprocess(ei.astype(np.int64))
    print("G_w:", struct["G_w"])
    print("K sums:", [int(k.sum()) for k in struct["K_w"]])
    print("widths:", struct["idx_e_w"], struct["idx_a_w"], struct["idx_c_w"])

